# revision 28
# baseline (speedup 1.0000x reference)
"""GateRow kernel for Trainium2 (8 NeuronCores, SPMD, gate-sharded, bit-packed).

Problem: out[b, g] = gates[g, 2*x[b, c0[g]] + x[b, c1[g]]]
  x: [16384, 8192] bool, gates: [8192, 4] bool, choices: [8192, 2] int32.

Strategy:
  Every 2-input boolean gate is  rowA OP rowB  for OP in {AND, OR, XOR}
  once operand inversion and constants are absorbed into a doubled
  lookup table TAB = [x^T ; ~x^T ; ones ; zeros] (one row per wire).
  Bit-pack the batch dimension (8 rows/byte) so each TAB row is
  B/8 = 2048 bytes and the boolean op is a plain bitwise op (done on
  uint32 views: bitwise is byte-local, and 32-bit elements quarter the
  DVE element count).

  Shard by GATES: core k owns 1024 gates.  Host sorts gates into
  type-homogeneous blocks of 128 (and / or / xor / copy families;
  "flexible" gates — constants and projections — pad the op families
  and fill pure-copy blocks that skip the ALU).

  Device critical path is SWDGE descriptor generation: each native
  indirect DMA gathers exactly 128 rows (one descriptor per partition,
  ~1.4us effective on the gpsimd queue), so every avoided gather call
  matters.  v4:
    - CONST block: constant gates (tt 0/15) are concentrated into one
      block per core, partition-split [ones | zeros] identically on
      every core; it is produced by two vector-engine memsets and
      DMAed out immediately — no gather, no table traffic, and it
      streams in the otherwise-idle early window.
    - idx table is DMAed by the sync engine as its first instruction
      (earliest slot after the preamble), and the SWDGE descriptor ring
      is enlarged to 128KB so all 1536 gather descriptors fit without
      mid-wall ring-space stalls.
    - every gather has its own SBUF tile (no pool-reuse waits); op
      blocks are gathered as [A_b, B_b] pairs with the vector op +
      per-block output DMA issued as soon as the pair lands; copy
      blocks go last (gathered tile DMAed straight out).
    - output DRAM is [128, NBLK*512] u32 partition-major so each block
      writes a plain 2D [128, 512] slice; the host folds the layout.
  The host un-permutes output columns.
"""

import sys

for _p in ("/opt/trn_rl_repo", "/opt/pypackages"):
    if _p not in sys.path:
        sys.path.append(_p)

from contextlib import ExitStack

import numpy as np

import concourse.bass as bass
import concourse.bacc as bacc
import concourse.tile as tile
import concourse.mybir as mybir
from concourse.bass_utils import run_bass_kernel_spmd

B, N, G, NCORES = 16384, 8192, 8192, 8
GPC = G // NCORES           # 1024 gates per core
NBLK = GPC // 128           # 8 gate blocks per core
PB = B // 8                 # 2048 packed bytes per table row
PW = PB // 4                # 512 packed uint32 words per table row
ROW_ONE = 2 * N             # all-ones table row
ROW_ZERO = 2 * N + 1        # all-zeros table row

# ---------------------------------------------------------------------------
# Gate classification.
#   tt bit (2a+b) = f(a, b).  Operand selectors:
#     0: x[c0]   1: ~x[c0]   2: x[c1]   3: ~x[c1]   4: ones   5: zeros
#   _SEL[op][tt] = (selA, selB) with f == rowA op rowB; None if
#   inexpressible.  _SEL["copy"][tt] = (selA, selA) when f == rowA.
# ---------------------------------------------------------------------------

_OPS = ("and", "or", "xor")
_NPOP = {"and": np.bitwise_and, "or": np.bitwise_or, "xor": np.bitwise_xor}


def _val(sel, a, b):
    return (a, 1 - a, b, 1 - b, 1, 0)[sel]


def _build_sel():
    sel = {op: [None] * 16 for op in (*_OPS, "copy")}
    for tt in range(16):
        for sa in range(6):
            if all(
                _val(sa, a, b) == ((tt >> (2 * a + b)) & 1)
                for a in (0, 1) for b in (0, 1)
            ):
                sel["copy"][tt] = (sa, sa)
                break
        for op in _OPS:
            for sa in range(6):
                for sb in range(6):
                    ok = all(
                        int(_NPOP[op](_val(sa, a, b), _val(sb, a, b)))
                        == ((tt >> (2 * a + b)) & 1)
                        for a in (0, 1) for b in (0, 1)
                    )
                    if ok and sel[op][tt] is None:
                        sel[op][tt] = (sa, sb)
    return sel


_SEL = _build_sel()
# Required family per tt: the single op that expresses it, or "copy".
_REQ = [
    "copy" if _SEL["copy"][tt] is not None
    else next(op for op in _OPS if _SEL[op][tt] is not None)
    for tt in range(16)
]


# ---------------------------------------------------------------------------
# Device program (parameterized by the per-core block schedule)
# ---------------------------------------------------------------------------

_ALU = {
    "and": mybir.AluOpType.bitwise_and,
    "or": mybir.AluOpType.bitwise_or,
    "xor": mybir.AluOpType.bitwise_xor,
}


def build_nc(key, ncores=NCORES):
    """One SPMD program; all cores run it on their own gate shard.

    key = (sched, k1): sched is a tuple of NBLK block kinds
    ("and"/"or"/"xor"/"copy"/"const"), op blocks first, copies next,
    const (if any) last; k1 = number of all-ones partitions in the
    const block.  idx column s feeds gather s.
    """
    sched, k1 = key
    ncalls = sum(2 if k in _OPS else (1 if k == "copy" else 0) for k in sched)

    nc = bacc.Bacc(
        "TRN2",
        target_bir_lowering=False,
        debug=False,
        num_devices=ncores,
        num_swdge_queues=1,
        dynamic_dma_scratch_size=2**17,
    )
    # per-core COMPACTED table: only the rows this core's gathers
    # reference (<= 128*ncalls), remapped on host.  10x smaller upload
    # than replicating the full doubled table, better DRAM locality.
    tab = nc.dram_tensor("tab", [128 * ncalls, PW], mybir.dt.uint32, kind="ExternalInput")
    idxs = nc.dram_tensor("idxs", [128, ncalls], mybir.dt.int32, kind="ExternalInput")
    # partition-major output: column-block bk holds partition-p rows of
    # device gate slot bk*128 + p
    outd = nc.dram_tensor("out", [128, NBLK * PW], mybir.dt.uint32, kind="ExternalOutput")

    # idx columns split across two engines/tiles (gpsimd stays free);
    # empirically this also keeps the gather wall free of the ~1.8us
    # ring-boundary stalls seen with a single idx DMA.
    na = (ncalls + 1) // 2
    splits = [s for s in (na, ncalls - na) if s > 0]

    with tile.TileContext(nc) as tc, ExitStack() as ctx:
        pool = ctx.enter_context(tc.tile_pool(name="p", bufs=1))

        idx_tiles = []          # (tile, first_col, ncols)
        c0 = 0
        for i, (ncols, eng) in enumerate(zip(splits, (nc.sync, nc.scalar))):
            t = pool.tile([128, ncols], mybir.dt.int32, tag=f"idx{i}", name=f"idx{i}")
            eng.dma_start(t[:], idxs[:, c0 : c0 + ncols])
            idx_tiles.append((t, c0, ncols))
            c0 += ncols

        def idx_col(s):
            for t, base, ncols in idx_tiles:
                if base <= s < base + ncols:
                    return t[:, s - base : s - base + 1]
            raise AssertionError(s)

        # const block: full-tile memsets on the idle vector engine
        # (partition-sliced memsets break walrus codegen) + two
        # partition-ranged output DMAs in the otherwise-empty early
        # window.
        if "const" in sched:
            bk = sched.index("const")
            osl = outd[:, bk * PW : (bk + 1) * PW]
            if k1 > 0:
                c1_t = pool.tile([128, PW], mybir.dt.uint32, tag="c1", name="c1_t")
                nc.vector.memset(c1_t[:], 0xFFFFFFFF)
                nc.sync.dma_start(osl[0:k1, :], c1_t[0:k1, :])
            if k1 < 128:
                c0_t = pool.tile([128, PW], mybir.dt.uint32, tag="c0", name="c0_t")
                nc.vector.memset(c0_t[:], 0)
                nc.scalar.dma_start(osl[k1:128, :], c0_t[0 : 128 - k1, :])

        def gather(s):
            g_t = pool.tile(
                [128, PW], mybir.dt.uint32, tag=f"g{s}", name=f"g{s}"
            )
            nc.gpsimd.indirect_dma_start(
                out=g_t[:],
                out_offset=None,
                in_=tab[:],
                in_offset=bass.IndirectOffsetOnAxis(ap=idx_col(s), axis=0),
            )
            return g_t

        out_engs = [nc.sync, nc.scalar]
        s = 0
        for bk, kind in enumerate(sched):
            if kind == "const":
                continue
            osl = outd[:, bk * PW : (bk + 1) * PW]
            eng = out_engs[bk % 2]
            if kind == "copy":
                a_t = gather(s)
                s += 1
                eng.dma_start(osl, a_t[:])
            else:
                a_t = gather(s)
                b_t = gather(s + 1)
                s += 2
                o_t = pool.tile(
                    [128, PW], mybir.dt.uint32, tag=f"o{bk}", name=f"o{bk}"
                )
                nc.vector.tensor_tensor(o_t[:], a_t[:], b_t[:], op=_ALU[kind])
                eng.dma_start(osl, o_t[:])
        assert s == ncalls
    nc.compile()
    return nc


# ---------------------------------------------------------------------------
# Host-side input prep
# ---------------------------------------------------------------------------


def _prep(x, gates, choices):
    x8 = np.asarray(x, dtype=np.uint8)
    gates8 = np.asarray(gates, dtype=np.uint8)
    ch = np.asarray(choices, dtype=np.int64)

    # Packed doubled table (replicated on every core).
    xp = np.packbits(x8, axis=0)              # [B/8, N], bit MSB = lowest batch row
    tab = np.empty((2 * N + 2, PB), dtype=np.uint8)
    tab[:N] = xp.T
    tab[N : 2 * N] = ~tab[:N]
    tab[ROW_ONE] = 0xFF
    tab[ROW_ZERO] = 0x00

    # Data-driven schedule: block counts from the actual type census.
    tt = (gates8 << np.arange(4, dtype=np.uint8)).sum(axis=1).astype(np.int64)
    req = np.array([_REQ[t] for t in range(16)])[tt]    # per-gate family
    gid = np.arange(G)
    nblk = {op: -(-int((req == op).sum()) // (128 * NCORES)) for op in _OPS}
    nop_blk = sum(nblk.values())

    # const block: k1 ones-partitions + k0 zeros-partitions per core,
    # identical split on every core (SPMD).  Falls back to 0 blocks if
    # the census can't fill it.
    ones = gid[tt == 15]
    zeros = gid[tt == 0]
    k1 = min(len(ones) // NCORES, 128)
    k0 = 128 - k1
    cblk = 1 if (len(zeros) >= NCORES * k0 and nop_blk + 1 <= NBLK) else 0
    if cblk == 0:
        k1 = 0
        k0 = 0
    bcopy = NBLK - nop_blk - cblk
    assert bcopy >= 0, f"schedule overflow: {nblk}"
    # Issue/layout order: op blocks first (vec+out stream under later
    # descriptor generation), copies last (shortest dependency tail: no
    # vector op), const last in layout (device issues it first; no
    # gather).
    sched = (
        sum(((op,) * nblk[op] for op in _OPS), ())
        + ("copy",) * bcopy
        + ("const",) * cblk
    )
    key = (sched, k1)
    cap = {op: nblk[op] * 128 * NCORES for op in _OPS}
    cap["copy"] = bcopy * 128 * NCORES

    # Pull const-block gates out first, then fill op buckets with their
    # required gates, pad with the remaining copy-capable gates; what's
    # left fills the copy blocks exactly.
    const_ones = ones[: NCORES * k1]
    const_zeros = zeros[: NCORES * k0]
    in_const = np.zeros(G, dtype=bool)
    in_const[const_ones] = True
    in_const[const_zeros] = True
    flex_pool = gid[(req == "copy") & ~in_const]
    fp = 0
    slots = {}
    for op in _OPS:
        need = gid[req == op]
        pad = cap[op] - len(need)
        assert pad >= 0
        slots[op] = np.concatenate([need, flex_pool[fp : fp + pad]])
        fp += pad
    slots["copy"] = flex_pool[fp:]
    assert len(slots["copy"]) == cap["copy"]

    # Device gate order (core-major, sched-order) + operand rows.
    npc = {"copy": bcopy * 128, "const": cblk * 128}
    for op in _OPS:
        npc[op] = nblk[op] * 128
    ncalls = sum(2 if k in _OPS else (1 if k == "copy" else 0) for k in sched)
    perm = np.empty(G, dtype=np.int64)        # device row -> gate id
    offs = np.empty((NCORES, 128, max(ncalls, 1)), dtype=np.int32)
    lut = {op: [_SEL[op][t] or (5, 5) for t in range(16)] for op in (*_OPS, "copy")}
    r = 0
    for k in range(NCORES):
        s = 0
        fam_pos = {op: 0 for op in (*_OPS, "copy")}
        for kind in sched:
            if kind == "const":
                # partitions [0,k1) ones, [k1,128) zeros
                perm[r : r + k1] = const_ones[k * k1 : (k + 1) * k1]
                perm[r + k1 : r + 128] = const_zeros[k * k0 : (k + 1) * k0]
                r += 128
                continue
            j = fam_pos[kind]
            fam_pos[kind] = j + 1
            g = slots[kind][k * npc[kind] + j * 128 : k * npc[kind] + (j + 1) * 128]
            selA = np.array([q[0] for q in lut[kind]])[tt[g]]
            selB = np.array([q[1] for q in lut[kind]])[tt[g]]
            rows = np.stack(
                [ch[g, 0], ch[g, 0] + N, ch[g, 1], ch[g, 1] + N,
                 np.full(len(g), ROW_ONE), np.full(len(g), ROW_ZERO)]
            )
            ar = np.arange(128)
            perm[r : r + 128] = g
            offs[k, :, s] = rows[selA, ar]
            s += 1
            if kind != "copy":
                offs[k, :, s] = rows[selB, ar]
                s += 1
            r += 128
        assert s == ncalls
    assert r == G

    # Compact each core's table to its referenced rows; remap indices.
    npad = 128 * ncalls
    in_maps = []
    for k in range(NCORES):
        refs = offs[k, :, :ncalls]
        uniq, inv = np.unique(refs, return_inverse=True)
        assert len(uniq) <= npad
        tabk = np.zeros((npad, PB), dtype=np.uint8)
        tabk[: len(uniq)] = tab[uniq]
        in_maps.append(
            {
                "tab": tabk.view(np.uint32),
                "idxs": np.ascontiguousarray(
                    inv.reshape(128, ncalls).astype(np.int32)
                ),
            }
        )
    return in_maps, perm, key


# ---------------------------------------------------------------------------
# Entry point
# ---------------------------------------------------------------------------

_NC_CACHE = {}


def _get_nc(key):
    if key not in _NC_CACHE:
        _NC_CACHE[key] = build_nc(key)
    return _NC_CACHE[key]


def _spot_check(out_bg, x8, gates8, ch, rng):
    """Verify ~64 random gate columns against host truth (catches the
    rare silent device corruption after an NRT fault)."""
    gs = rng.choice(G, size=64, replace=False)
    a = x8[:, ch[gs, 0]].astype(np.int32)
    b = x8[:, ch[gs, 1]].astype(np.int32)
    exp = gates8[gs[None, :], a * 2 + b].astype(bool)   # [B, 64]
    return bool((out_bg[:, gs] == exp).all())


def kernel(x, gates, choices):
    in_maps, perm, key = _prep(x, gates, choices)
    nc = _get_nc(key)
    x8 = np.asarray(x, dtype=np.uint8)
    gates8 = np.asarray(gates, dtype=np.uint8)
    ch = np.asarray(choices, dtype=np.int64)
    rng = np.random.default_rng(0)
    last = None
    for attempt in range(4):
        try:
            res = run_bass_kernel_spmd(nc, in_maps, list(range(NCORES)))
        except Exception:
            # Transient axon/NRT INTERNAL errors occur occasionally; a
            # pause + retry recovers them.
            import time

            time.sleep(10)
            continue
        packed = np.concatenate(
            [
                # [128, NBLK*2048]u8 -> block-major [1024, 2048]
                res.results[k]["out"].view(np.uint8)
                .reshape(128, NBLK, PB).transpose(1, 0, 2).reshape(GPC, PB)
                for k in range(NCORES)
            ],
            axis=0,
        )
        ordered = np.empty_like(packed)
        ordered[perm] = packed                # un-permute gate rows
        up = np.unpackbits(ordered, axis=1)   # [G, B] 0/1 uint8
        last = up.view(np.bool_).T            # [B, G] bool view
        if _spot_check(last, x8, gates8, ch, rng):
            return last
        # Silent device corruption (seen once after an NRT fault):
        # rerun; the next execution recovers.
    return last
